# revision 23
# baseline (speedup 1.0000x reference)
"""AKOrN layer on 8 TRN2 NeuronCores, data-parallel over batch.

reference: v = l2norm_d(x @ W_in); K = tanh(coupling);
           8x: v = l2norm_d(v + K @ v + omega); return v [B, OUT, D]

Implementation notes:
- Data-parallel: batch 8192 -> 1024 rows per core; W_in/coupling/omega
  replicated. No collectives.
- K' = tanh(coupling) + I folds the "+ v" into the step matmul, so each step
  is pure matmul work plus a PSUM-side normalize.
- v lives on-chip as 4 per-d planes [OUT(part), batch(free)] in fp16 (8x
  finer mantissa than bf16 at identical PE speed; bf16 landed at rel err
  2.4e-2, fp16 at 3.3e-3). Batch is processed in 2 sequential 512-column
  chunks (SBUF fit for the double-buffered v generations).
- Step: 8 j-tiles x 4 d matmuls accumulate K'^T @ v_d into 4 PSUM banks
  (2 normalize units in flight = all 8 banks), then:
    q_d = Square(psum_d + omega_d)        (ACT, per-partition bias, fp16 out)
    s   = q0+q1+q2+q3                     (DVE fp16, 2x mode)
    inv = Exp(-0.5 * Ln(s))               (ACT, == rsqrt(s), one act table)
    v'_d = (psum_d + omega_d) * inv       (DVE scalar_tensor_tensor -> fp16)
- Last step runs transposed (stationary = v-slice, moving = K'^T rows) to
  produce [batch, OUT] so the d-interleave + output DMA is contiguous;
  omega enters there via a K=1 ones-row matmul (skipped when omega == 0).
  u is staged out of PSUM by ACT copies so banks free early.
- chunk1's first matmul is emitted between chunk0's steps and last step:
  its matmuls fill the last-step PSUM-drain stalls.
- x^T and W_in are uploaded pre-transposed/de-interleaved in fp16 (host-side
  layout marshalling only; all model arithmetic runs on device).
- Tile pre-splits every Matmult into Ldweights+Matmult; a BIR-JSON post-pass
  drops Ldweights that reload the identical stationary (the d-loop reuses
  each K' tile 4x), and bacc's act-table pass is disabled in favor of
  walrus lower_act (bacc's greedy alternated two tables 289x per kernel).
Measured: ~1.06 ms HW exec, rel err 3.3e-3 (gate 2e-2); PE union busy ~95%
of span; 4608 matmuls x ~220 ns vs 213 ns streaming floor.
"""
import contextlib
import ctypes
import os
import sys
import types

import numpy as np

B, IN, OUT, D = 8192, 1024, 1024, 4
STEPS = 8
NCORES = 8
BS = B // NCORES      # batch shard per core = 1024
CH = 512              # on-chip batch chunk (2 chunks, processed serially)
NCH = BS // CH
P = 128
NT = OUT // P         # 8 partition tiles

_SO_PATH = "/opt/axon/libaxon_pjrt.so"


# ---------------------------------------------------------------- plumbing
def _ntff_profile_via_ctypes(so_path):
    try:
        lib = ctypes.CDLL(so_path)
    except OSError:
        return None
    if not hasattr(lib, "axon_start_nrt_profile"):
        return None
    lib.axon_start_nrt_profile.argtypes = [ctypes.POINTER(ctypes.c_int64), ctypes.c_size_t]
    lib.axon_start_nrt_profile.restype = ctypes.c_int64
    lib.axon_stop_nrt_profile.argtypes = [ctypes.c_char_p]
    lib.axon_stop_nrt_profile.restype = ctypes.c_int64

    @contextlib.contextmanager
    def _hook(output_dir, device_ids):
        import jax

        jax.devices()
        if device_ids:
            ids = (ctypes.c_int64 * len(device_ids))(*device_ids)
            rc = lib.axon_start_nrt_profile(ids, len(device_ids))
        else:
            rc = lib.axon_start_nrt_profile(None, 0)
        if rc != 0:
            raise RuntimeError(f"axon_start_nrt_profile rc={rc}")
        try:
            yield
        finally:
            n = lib.axon_stop_nrt_profile(str(output_dir).encode())
            print(f"profile: {n} file(s) written to {output_dir}", file=sys.stderr)

    return _hook


def _install_hook_shim():
    if "antenv.axon_hooks" in sys.modules:
        return
    try:
        import antenv
    except ImportError:
        return
    mod = types.ModuleType("antenv.axon_hooks")
    _state = {"hook": _ntff_profile_via_ctypes(_SO_PATH)}
    mod.set_axon_ntff_profile_hook = lambda h: _state.__setitem__("hook", h)
    mod.get_axon_ntff_profile_hook = lambda: _state["hook"]
    sys.modules["antenv.axon_hooks"] = mod
    antenv.axon_hooks = mod


def _patch_ldw_opt():
    import concourse.bass_utils as bu

    if os.environ.get("KERNEL_FUSE") != "1":
        return
    if getattr(bu, "_ldw_patched", False):
        return
    orig = bu.run_command

    def patched(argv, **kwargs):
        argv = [
            a.replace("--enable-ldw-opt=false", "--enable-ldw-opt=true")
            if isinstance(a, str)
            else a
            for a in argv
        ]
        return orig(argv, **kwargs)

    bu.run_command = patched
    bu._ldw_patched = True


def _dedup_ldweights_json(nc):
    """Drop Ldweights that reload the exact weights already resident
    (identical operand AP as previous Ldweights, only Matmults between)."""
    import orjson

    orig = nc.to_json_bytes

    def patched():
        bir = orjson.loads(orig())
        n = 0
        for f in bir.get("functions", []):
            for blk in f.get("blocks", []):
                insts = blk.get("instructions")
                if not insts:
                    continue
                keep = []
                last_w = None
                for ins in insts:
                    op = ins.get("opcode")
                    if op == "Ldweights":
                        si = ins.get("sync_info") or {}
                        wts = si.get("on_wait") or []
                        key = orjson.dumps(ins.get("ins"))
                        if key == last_w and not wts:
                            n += 1
                            continue
                        last_w = key
                        keep.append(ins)
                    elif op == "Matmult":
                        keep.append(ins)
                    else:
                        last_w = None
                        keep.append(ins)
                blk["instructions"] = keep
        return orjson.dumps(bir)

    nc.to_json_bytes = patched
    return nc


def _fuse_ldweights_json(nc):
    import orjson

    orig = nc.to_json_bytes

    def patched():
        bir = orjson.loads(orig())
        for f in bir.get("functions", []):
            for blk in f.get("blocks", []):
                insts = blk.get("instructions")
                if not insts:
                    continue
                keep = []
                for ins in insts:
                    if ins.get("opcode") == "Ldweights":
                        si = ins.get("sync_info") or {}
                        w = si.get("on_wait") or []
                        assert not (si.get("on_update") or []), ins["name"]
                        if w:
                            keep.append({
                                "opcode": "NoOp",
                                "name": ins["name"],
                                "engine": ins.get("engine", "PE"),
                                "ins": [],
                                "outs": [],
                                "sync_info": {"on_wait": w, "on_update": []},
                            })
                        continue
                    keep.append(ins)
                blk["instructions"] = keep
        return orjson.dumps(bir)

    nc.to_json_bytes = patched
    return nc


# ---------------------------------------------------------------- builder
def _build(omega_zero):
    import concourse.bacc as bacc
    import concourse.mybir as mybir
    from concourse import tile

    A = mybir.ActivationFunctionType
    Op = mybir.AluOpType
    f32 = mybir.dt.float32
    bf16 = mybir.dt.float16  # fp16: 8x finer mantissa than bf16, same PE speed

    class BaccNoSplit(bacc.Bacc):
        def move_matmul_waits_to_ldweights(self):
            return

        def insert_act_table_loads(self):
            # walrus lower_act picks act-func sets globally (bacc's greedy
            # alternates natural_log/exp_and_others per normalize unit,
            # 289 table reloads)
            return

    nc = BaccNoSplit(None, target_bir_lowering=False)

    xT = nc.declare_dram_parameter("xT", [IN, BS], mybir.dt.float16, isOutput=False)
    w = nc.declare_dram_parameter("w", [D, IN, OUT], mybir.dt.float16, isOutput=False)
    ct = nc.declare_dram_parameter("ct", [OUT, OUT], f32, isOutput=False)
    om = nc.declare_dram_parameter("om", [OUT, D], f32, isOutput=False)
    omr = nc.declare_dram_parameter("omr", [D, OUT], f32, isOutput=False)
    eye_in = nc.declare_dram_parameter("eye_in", [P, P], f32, isOutput=False)
    out = nc.declare_dram_parameter("out", [BS, OUT * D], f32, isOutput=True)

    with tile.TileContext(nc) as tc, contextlib.ExitStack() as ctx:
        const = ctx.enter_context(tc.tile_pool(name="const", bufs=1))
        pool = ctx.enter_context(tc.tile_pool(name="pool", bufs=1))
        psum = ctx.enter_context(tc.tile_pool(name="psum", bufs=1, space="PSUM"))

        omc = []
        omrb = []
        ktb = []
        ones1_box = []

        def emit_preamble():
            # constants / K' = tanh(ct)^T rows + I. Emitted AFTER chunk 0's
            # first matmul so the PE starts as soon as xt/wb land.
            eyef = const.tile([P, P], f32, name="eyef", tag="eyef")
            nc.sync.dma_start(eyef[:], eye_in[:])
            eyeb = const.tile([P, P], bf16, name="eyeb", tag="eyeb")
            nc.vector.tensor_copy(eyeb[:], eyef[:])

            for t in range(NT):  # omega columns per o-tile: [128, D] fp32
                o = const.tile([P, D], f32, name=f"omc{t}", tag=f"omc{t}")
                nc.sync.dma_start(o[:], om[t * P:(t + 1) * P, :])
                omc.append(o)

            if not omega_zero:
                for d in range(D):  # omega rows per d: [1, OUT] fp16
                    of = const.tile([1, OUT], f32, name=f"omrf{d}", tag=f"omrf{d}")
                    nc.sync.dma_start(of[:], omr[d:d + 1, :])
                    ob = const.tile([1, OUT], bf16, name=f"omrb{d}", tag=f"omrb{d}")
                    nc.vector.tensor_copy(ob[:], of[:])
                    omrb.append(ob)
                ones1 = const.tile([1, P], bf16, name="ones1", tag="ones1")
                nc.gpsimd.memset(ones1[:], 1.0)
                ones1_box.append(ones1)

            for j in range(NT):  # K'^T tiles: [128(j), OUT(i)] fp16
                kf = pool.tile([P, OUT], f32, name=f"ktf{j}", tag="ktf", bufs=2)
                nc.sync.dma_start(kf[:], ct[j * P:(j + 1) * P, :])
                kb = const.tile([P, OUT], bf16, name=f"ktb{j}", tag=f"ktb{j}")
                nc.scalar.activation(kb[:], kf[:], A.Tanh)
                nc.vector.tensor_tensor(
                    kb[:, j * P:(j + 1) * P], kb[:, j * P:(j + 1) * P], eyeb[:],
                    op=Op.add,
                )
                ktb.append(kb)

        # v planes: tag per (d, j), 2 bufs (generation ping-pong)
        def v_tile(d, j, s):
            return const.tile([P, CH], bf16, name=f"v_s{s}_d{d}_j{j}",
                             tag=f"v{d}_{j}", bufs=2)

        def normalize_unit(ps, bias_aps, vout, n, label):
            """ps: 4 psum APs [P,n]; bias_aps: 4 per-part scalars or None;
            vout(d, inv) -> emits the final scaled write for plane d."""
            q = [pool.tile([P, n], bf16, name=f"q{d}_{label}", tag=f"q{d}", bufs=2)
                 for d in range(D)]
            for d in range(D):
                if bias_aps is None:
                    nc.scalar.activation(q[d][:], ps[d], A.Square)
                else:
                    nc.scalar.activation(q[d][:], ps[d], A.Square, bias=bias_aps[d])
            s01 = pool.tile([P, n], bf16, name=f"s01_{label}", tag="s01", bufs=2)
            s23 = pool.tile([P, n], bf16, name=f"s23_{label}", tag="s23", bufs=2)
            ssum = pool.tile([P, n], bf16, name=f"ss_{label}", tag="ss", bufs=2)
            nc.vector.tensor_tensor(s01[:], q[0][:], q[1][:], op=Op.add)
            nc.vector.tensor_tensor(s23[:], q[2][:], q[3][:], op=Op.add)
            nc.vector.tensor_tensor(ssum[:], s01[:], s23[:], op=Op.add)
            lns = pool.tile([P, n], f32, name=f"ln_{label}", tag="lns", bufs=2)
            nc.scalar.activation(lns[:], ssum[:], A.Ln)
            inv = pool.tile([P, n], f32, name=f"inv_{label}", tag="inv", bufs=2)
            nc.scalar.activation(inv[:], lns[:], A.Exp, scale=-0.5)
            for d in range(D):
                vout(d, inv)

        def first_matmul(c):
            # v0 = l2norm(x @ W_in)
            xt = pool.tile([P, NT * CH], bf16, name=f"xt{c}", tag="xt", bufs=2)
            nc.sync.dma_start(
                xt[:].rearrange("p (t n) -> p t n", t=NT),
                xT.rearrange("(t p) b -> p t b", p=P)[:, :, c * CH:(c + 1) * CH],
            )

            vcur = {}
            for ot in range(NT):
                wb = []
                for d in range(D):
                    wbd = pool.tile([P, OUT], bf16, name=f"wb{c}_{ot}_{d}",
                                    tag="wb", bufs=6)
                    nc.sync.dma_start(
                        wbd[:].rearrange("p (t n) -> p t n", t=NT),
                        w.rearrange("d (t p) o -> d p t o", p=P)[d, :, :, ot * P:(ot + 1) * P],
                    )
                    wb.append(wbd)

                ps = [psum.tile([P, CH], f32, name=f"ps0_{c}_{ot}_{d}",
                                tag=f"ps{d}", bufs=2) for d in range(D)]
                for it in range(NT):
                    for d in range(D):
                        nc.tensor.matmul(
                            ps[d][:],
                            wb[d][:, it * P:(it + 1) * P],
                            xt[:, it * CH:(it + 1) * CH],
                            start=(it == 0), stop=(it == NT - 1),
                        )

                def vout0(d, inv, _ot=ot, _c=c):
                    vt = v_tile(d, _ot, 0)
                    vcur.setdefault(d, {})[_ot] = vt
                    nc.vector.tensor_tensor(vt[:], ps[d][:], inv[:], op=Op.mult)

                normalize_unit([p[:] for p in ps], None, vout0, CH, f"f{c}_{ot}")
            return vcur

        # PE warm-up: ~5us of dummy matmuls on memset data fill the initial
        # input-DMA wait and lift the HAM clock gate to 2.4 GHz before the
        # first real matmul issues.
        wuw = pool.tile([P, P], bf16, name="wuw", tag="wuw")
        wux = pool.tile([P, 512], bf16, name="wux", tag="wux")
        nc.gpsimd.memset(wuw[:], 0.0)
        nc.gpsimd.memset(wux[:], 0.0)
        wups = psum.tile([P, 512], f32, name="wups", tag="ps0", bufs=2)
        for _ in range(24):
            nc.tensor.matmul(wups[:], wuw[:], wux[:], start=True, stop=True)

        vcur_pending = {0: first_matmul(0)}
        emit_preamble()

        for c in range(NCH):
            vcur = vcur_pending.pop(c)
            # ---------------- steps 1..STEPS-1 (normal orientation) ------
            for s in range(1, STEPS):
                vnext = {}
                for it in range(NT):
                    ps = [psum.tile([P, CH], f32, name=f"ps{s}_{c}_{it}_{d}",
                                    tag=f"ps{d}", bufs=2) for d in range(D)]
                    for j in range(NT):
                        for d in range(D):
                            nc.tensor.matmul(
                                ps[d][:],
                                ktb[j][:, it * P:(it + 1) * P],
                                vcur[d][j][:],
                                start=(j == 0), stop=(j == NT - 1),
                            )
                    bias_aps = [omc[it][:, d:d + 1] for d in range(D)]

                    def vouts(d, inv, _it=it, _s=s):
                        vt = v_tile(d, _it, _s)
                        vnext.setdefault(d, {})[_it] = vt
                        nc.vector.scalar_tensor_tensor(
                            vt[:], ps[d][:], omc[_it][:, d:d + 1], inv[:],
                            op0=Op.add, op1=Op.mult,
                        )

                    normalize_unit([p[:] for p in ps], bias_aps, vouts, CH,
                                   f"s{s}_{c}_{it}")
                vcur = vnext

            # chunk c+1's first matmul emitted here: its matmuls fill the
            # last-step drain stalls, and its v0 slots are free by now.
            if c + 1 < NCH:
                vcur_pending[c + 1] = first_matmul(c + 1)

            # ---------------- last step, transposed: u[b, i] --------------
            for bt in range(CH // P):
                for ic in range(2):
                    V = pool.tile([P, 512 * D], f32, name=f"V{c}_{bt}_{ic}",
                                  tag="big", bufs=3)
                    ps = [psum.tile([P, 512], f32, name=f"psL_{c}_{bt}_{ic}_{d}",
                                    tag=f"ps{d}", bufs=2) for d in range(D)]
                    for j in range(NT):
                        for d in range(D):
                            nc.tensor.matmul(
                                ps[d][:],
                                vcur[d][j][:, bt * P:(bt + 1) * P],
                                ktb[j][:, ic * 512:(ic + 1) * 512],
                                start=(j == 0),
                                stop=(omega_zero and j == NT - 1),
                            )
                    if not omega_zero:
                        for d in range(D):
                            nc.tensor.matmul(
                                ps[d][:],
                                ones1_box[0][:],
                                omrb[d][:, ic * 512:(ic + 1) * 512],
                                start=False, stop=True,
                            )

                    # stage u out of PSUM early so the banks free in ~3us
                    # instead of being held through the slow strided V writes
                    u = [pool.tile([P, 512], f32, name=f"u{d}_L{c}_{bt}_{ic}",
                                   tag=f"u{d}", bufs=2) for d in range(D)]
                    for d in range(D):
                        nc.scalar.copy(u[d][:], ps[d][:])

                    def voutL(d, inv, _V=V, _u=u):
                        nc.vector.tensor_tensor(
                            _V[:, d:d + 511 * D + 1:D], _u[d][:], inv[:], op=Op.mult
                        )

                    normalize_unit([p[:] for p in ps], None, voutL, 512,
                                   f"L{c}_{bt}_{ic}")
                    nc.sync.dma_start(
                        out[(c * CH + bt * P):(c * CH + (bt + 1) * P),
                            512 * D * ic:512 * D * (ic + 1)],
                        V[:],
                    )

    nc.finalize()
    if os.environ.get("KERNEL_FUSE") == "1":
        _fuse_ldweights_json(nc)
    elif os.environ.get("KERNEL_NODEDUP") != "1":
        _dedup_ldweights_json(nc)
    return nc


_CACHED = {}


def kernel(x, W_in, omega, coupling):
    _install_hook_shim()
    _patch_ldw_opt()
    from concourse.bass_utils import run_bass_kernel_spmd

    x = np.ascontiguousarray(np.asarray(x, dtype=np.float32))
    W_in = np.asarray(W_in, dtype=np.float32)
    omega = np.ascontiguousarray(np.asarray(omega, dtype=np.float32))
    coupling = np.asarray(coupling, dtype=np.float32)

    w_host = np.ascontiguousarray(W_in.transpose(2, 0, 1).astype(np.float16))  # [D, IN, OUT]
    ct_host = np.ascontiguousarray(coupling.T)                     # [OUT, OUT]
    omr_host = np.ascontiguousarray(omega.T)                       # [D, OUT]
    eye_host = np.eye(P, dtype=np.float32)

    omega_zero = not np.any(omega)
    key = ("nc", omega_zero)
    if key not in _CACHED:
        _CACHED[key] = _build(omega_zero)
    nc = _CACHED[key]

    in_maps = []
    for core in range(NCORES):
        xs = x[core * BS:(core + 1) * BS, :]
        in_maps.append({
            "xT": np.ascontiguousarray(xs.T.astype(np.float16)),
            "w": w_host,
            "ct": ct_host,
            "om": omega,
            "omr": omr_host,
            "eye_in": eye_host,
        })

    trace = os.environ.get("KERNEL_TRACE") == "1"
    res = run_bass_kernel_spmd(nc, in_maps, core_ids=list(range(NCORES)), trace=trace)
    if trace and res.exec_time_ns:
        print(f"HW exec time: {res.exec_time_ns} ns")
        _CACHED["exec_time_ns"] = res.exec_time_ns
        _CACHED["results"] = res

    outs = [res.results[i]["out"].reshape(BS, OUT, D) for i in range(NCORES)]
    return np.concatenate(outs, axis=0)


# revision 24
# speedup vs baseline: 1.0093x; 1.0093x over previous
"""AKOrN layer on 8 TRN2 NeuronCores, data-parallel over batch.

reference: v = l2norm_d(x @ W_in); K = tanh(coupling);
           8x: v = l2norm_d(v + K @ v + omega); return v [B, OUT, D]

Implementation notes:
- Data-parallel: batch 8192 -> 1024 rows per core; W_in/coupling/omega
  replicated. No collectives.
- K' = tanh(coupling) + I folds the "+ v" into the step matmul, so each step
  is pure matmul work plus a PSUM-side normalize.
- v lives on-chip as 4 per-d planes [OUT(part), batch(free)] in fp16 (8x
  finer mantissa than bf16 at identical PE speed; bf16 landed at rel err
  2.4e-2, fp16 at 3.3e-3). Batch is processed in 2 sequential 512-column
  chunks (SBUF fit for the double-buffered v generations).
- Step: 8 j-tiles x 4 d matmuls accumulate K'^T @ v_d into 4 PSUM banks
  (2 normalize units in flight = all 8 banks), then:
    q_d = Square(psum_d + omega_d)        (ACT, per-partition bias, fp16 out)
    s   = q0+q1+q2+q3                     (DVE fp16, 2x mode)
    inv = Exp(-0.5 * Ln(s))               (ACT, == rsqrt(s), one act table)
    v'_d = (psum_d + omega_d) * inv       (DVE scalar_tensor_tensor -> fp16)
- Last step runs transposed (stationary = v-slice, moving = K'^T rows) to
  produce [batch, OUT] so the d-interleave + output DMA is contiguous;
  omega enters there via a K=1 ones-row matmul (skipped when omega == 0).
  u is staged out of PSUM by ACT copies so banks free early.
- chunk1's first matmul is emitted between chunk0's steps and last step:
  its matmuls fill the last-step PSUM-drain stalls.
- x^T and W_in are uploaded pre-transposed/de-interleaved in fp16 (host-side
  layout marshalling only; all model arithmetic runs on device).
- Tile pre-splits every Matmult into Ldweights+Matmult; a BIR-JSON post-pass
  drops Ldweights that reload the identical stationary (the d-loop reuses
  each K' tile 4x), and bacc's act-table pass is disabled in favor of
  walrus lower_act (bacc's greedy alternated two tables 289x per kernel).
Measured: ~1.06 ms HW exec, rel err 3.3e-3 (gate 2e-2); PE union busy ~95%
of span; 4608 matmuls x ~220 ns vs 213 ns streaming floor.
"""
import contextlib
import ctypes
import os
import sys
import types

import numpy as np

B, IN, OUT, D = 8192, 1024, 1024, 4
STEPS = 8
NCORES = 8
BS = B // NCORES      # batch shard per core = 1024
CH = 512              # on-chip batch chunk (2 chunks, processed serially)
NCH = BS // CH
P = 128
NT = OUT // P         # 8 partition tiles

_SO_PATH = "/opt/axon/libaxon_pjrt.so"


# ---------------------------------------------------------------- plumbing
def _ntff_profile_via_ctypes(so_path):
    try:
        lib = ctypes.CDLL(so_path)
    except OSError:
        return None
    if not hasattr(lib, "axon_start_nrt_profile"):
        return None
    lib.axon_start_nrt_profile.argtypes = [ctypes.POINTER(ctypes.c_int64), ctypes.c_size_t]
    lib.axon_start_nrt_profile.restype = ctypes.c_int64
    lib.axon_stop_nrt_profile.argtypes = [ctypes.c_char_p]
    lib.axon_stop_nrt_profile.restype = ctypes.c_int64

    @contextlib.contextmanager
    def _hook(output_dir, device_ids):
        import jax

        jax.devices()
        if device_ids:
            ids = (ctypes.c_int64 * len(device_ids))(*device_ids)
            rc = lib.axon_start_nrt_profile(ids, len(device_ids))
        else:
            rc = lib.axon_start_nrt_profile(None, 0)
        if rc != 0:
            raise RuntimeError(f"axon_start_nrt_profile rc={rc}")
        try:
            yield
        finally:
            n = lib.axon_stop_nrt_profile(str(output_dir).encode())
            print(f"profile: {n} file(s) written to {output_dir}", file=sys.stderr)

    return _hook


def _install_hook_shim():
    if "antenv.axon_hooks" in sys.modules:
        return
    try:
        import antenv
    except ImportError:
        return
    mod = types.ModuleType("antenv.axon_hooks")
    _state = {"hook": _ntff_profile_via_ctypes(_SO_PATH)}
    mod.set_axon_ntff_profile_hook = lambda h: _state.__setitem__("hook", h)
    mod.get_axon_ntff_profile_hook = lambda: _state["hook"]
    sys.modules["antenv.axon_hooks"] = mod
    antenv.axon_hooks = mod


def _patch_ldw_opt():
    import concourse.bass_utils as bu

    if os.environ.get("KERNEL_FUSE") != "1":
        return
    if getattr(bu, "_ldw_patched", False):
        return
    orig = bu.run_command

    def patched(argv, **kwargs):
        argv = [
            a.replace("--enable-ldw-opt=false", "--enable-ldw-opt=true")
            if isinstance(a, str)
            else a
            for a in argv
        ]
        return orig(argv, **kwargs)

    bu.run_command = patched
    bu._ldw_patched = True


def _dedup_ldweights_json(nc):
    """Drop Ldweights that reload the exact weights already resident
    (identical operand AP as previous Ldweights, only Matmults between)."""
    import orjson

    orig = nc.to_json_bytes

    def patched():
        bir = orjson.loads(orig())
        n = 0
        for f in bir.get("functions", []):
            for blk in f.get("blocks", []):
                insts = blk.get("instructions")
                if not insts:
                    continue
                keep = []
                last_w = None
                for ins in insts:
                    op = ins.get("opcode")
                    if op == "Ldweights":
                        si = ins.get("sync_info") or {}
                        wts = si.get("on_wait") or []
                        key = orjson.dumps(ins.get("ins"))
                        if key == last_w and not wts:
                            n += 1
                            continue
                        last_w = key
                        keep.append(ins)
                    elif op == "Matmult":
                        keep.append(ins)
                    else:
                        last_w = None
                        keep.append(ins)
                blk["instructions"] = keep
        return orjson.dumps(bir)

    nc.to_json_bytes = patched
    return nc


def _fuse_ldweights_json(nc):
    import orjson

    orig = nc.to_json_bytes

    def patched():
        bir = orjson.loads(orig())
        for f in bir.get("functions", []):
            for blk in f.get("blocks", []):
                insts = blk.get("instructions")
                if not insts:
                    continue
                keep = []
                for ins in insts:
                    if ins.get("opcode") == "Ldweights":
                        si = ins.get("sync_info") or {}
                        w = si.get("on_wait") or []
                        assert not (si.get("on_update") or []), ins["name"]
                        if w:
                            keep.append({
                                "opcode": "NoOp",
                                "name": ins["name"],
                                "engine": ins.get("engine", "PE"),
                                "ins": [],
                                "outs": [],
                                "sync_info": {"on_wait": w, "on_update": []},
                            })
                        continue
                    keep.append(ins)
                blk["instructions"] = keep
        return orjson.dumps(bir)

    nc.to_json_bytes = patched
    return nc


# ---------------------------------------------------------------- builder
def _build(omega_zero):
    import concourse.bacc as bacc
    import concourse.mybir as mybir
    from concourse import tile

    A = mybir.ActivationFunctionType
    Op = mybir.AluOpType
    f32 = mybir.dt.float32
    bf16 = mybir.dt.float16  # fp16: 8x finer mantissa than bf16, same PE speed

    class BaccNoSplit(bacc.Bacc):
        def move_matmul_waits_to_ldweights(self):
            return

        def insert_act_table_loads(self):
            # walrus lower_act picks act-func sets globally (bacc's greedy
            # alternates natural_log/exp_and_others per normalize unit,
            # 289 table reloads)
            return

    nc = BaccNoSplit(None, target_bir_lowering=False)

    xT = nc.declare_dram_parameter("xT", [IN, BS], mybir.dt.float16, isOutput=False)
    w = nc.declare_dram_parameter("w", [D, IN, OUT], mybir.dt.float16, isOutput=False)
    ct = nc.declare_dram_parameter("ct", [OUT, OUT], f32, isOutput=False)
    om = nc.declare_dram_parameter("om", [OUT, D], f32, isOutput=False)
    omr = nc.declare_dram_parameter("omr", [D, OUT], f32, isOutput=False)
    eye_in = nc.declare_dram_parameter("eye_in", [P, P], f32, isOutput=False)
    out = nc.declare_dram_parameter("out", [BS, OUT * D], f32, isOutput=True)

    with tile.TileContext(nc) as tc, contextlib.ExitStack() as ctx:
        const = ctx.enter_context(tc.tile_pool(name="const", bufs=1))
        pool = ctx.enter_context(tc.tile_pool(name="pool", bufs=1))
        psum = ctx.enter_context(tc.tile_pool(name="psum", bufs=1, space="PSUM"))

        omc = []
        omrb = []
        ktb = []
        ones1_box = []

        def emit_preamble():
            # constants / K' = tanh(ct)^T rows + I. Emitted AFTER chunk 0's
            # first matmul so the PE starts as soon as xt/wb land.
            eyef = const.tile([P, P], f32, name="eyef", tag="eyef")
            nc.sync.dma_start(eyef[:], eye_in[:])
            eyeb = const.tile([P, P], bf16, name="eyeb", tag="eyeb")
            nc.vector.tensor_copy(eyeb[:], eyef[:])

            for t in range(NT):  # omega columns per o-tile: [128, D] fp32
                o = const.tile([P, D], f32, name=f"omc{t}", tag=f"omc{t}")
                nc.sync.dma_start(o[:], om[t * P:(t + 1) * P, :])
                omc.append(o)

            if not omega_zero:
                for d in range(D):  # omega rows per d: [1, OUT] fp16
                    of = const.tile([1, OUT], f32, name=f"omrf{d}", tag=f"omrf{d}")
                    nc.sync.dma_start(of[:], omr[d:d + 1, :])
                    ob = const.tile([1, OUT], bf16, name=f"omrb{d}", tag=f"omrb{d}")
                    nc.vector.tensor_copy(ob[:], of[:])
                    omrb.append(ob)
                ones1 = const.tile([1, P], bf16, name="ones1", tag="ones1")
                nc.gpsimd.memset(ones1[:], 1.0)
                ones1_box.append(ones1)

            for j in range(NT):  # K'^T tiles: [128(j), OUT(i)] fp16
                kf = pool.tile([P, OUT], f32, name=f"ktf{j}", tag="ktf", bufs=2)
                nc.sync.dma_start(kf[:], ct[j * P:(j + 1) * P, :])
                kb = const.tile([P, OUT], bf16, name=f"ktb{j}", tag=f"ktb{j}")
                nc.scalar.activation(kb[:], kf[:], A.Tanh)
                nc.vector.tensor_tensor(
                    kb[:, j * P:(j + 1) * P], kb[:, j * P:(j + 1) * P], eyeb[:],
                    op=Op.add,
                )
                ktb.append(kb)

        # v planes: tag per (d, j), 2 bufs (generation ping-pong)
        def v_tile(d, j, s):
            return const.tile([P, CH], bf16, name=f"v_s{s}_d{d}_j{j}",
                             tag=f"v{d}_{j}", bufs=2)

        def normalize_unit(ps, bias_aps, vout, n, label):
            """ps: 4 psum APs [P,n]; bias_aps: 4 per-part scalars or None;
            vout(d, inv) -> emits the final scaled write for plane d."""
            q = [pool.tile([P, n], bf16, name=f"q{d}_{label}", tag=f"q{d}", bufs=2)
                 for d in range(D)]
            for d in range(D):
                if bias_aps is None:
                    nc.scalar.activation(q[d][:], ps[d], A.Square)
                else:
                    nc.scalar.activation(q[d][:], ps[d], A.Square, bias=bias_aps[d])
            s01 = pool.tile([P, n], bf16, name=f"s01_{label}", tag="s01", bufs=2)
            s23 = pool.tile([P, n], bf16, name=f"s23_{label}", tag="s23", bufs=2)
            ssum = pool.tile([P, n], bf16, name=f"ss_{label}", tag="ss", bufs=2)
            nc.vector.tensor_tensor(s01[:], q[0][:], q[1][:], op=Op.add)
            nc.vector.tensor_tensor(s23[:], q[2][:], q[3][:], op=Op.add)
            nc.vector.tensor_tensor(ssum[:], s01[:], s23[:], op=Op.add)
            lns = pool.tile([P, n], f32, name=f"ln_{label}", tag="lns", bufs=2)
            nc.scalar.activation(lns[:], ssum[:], A.Ln)
            inv = pool.tile([P, n], f32, name=f"inv_{label}", tag="inv", bufs=2)
            nc.scalar.activation(inv[:], lns[:], A.Exp, scale=-0.5)
            for d in range(D):
                vout(d, inv)

        def first_matmul(c):
            # v0 = l2norm(x @ W_in)
            xt = pool.tile([P, NT * CH], bf16, name=f"xt{c}", tag="xt", bufs=2)
            nc.sync.dma_start(
                xt[:].rearrange("p (t n) -> p t n", t=NT),
                xT.rearrange("(t p) b -> p t b", p=P)[:, :, c * CH:(c + 1) * CH],
            )

            vcur = {}
            for ot in range(NT):
                wb = []
                for d in range(D):
                    wbd = pool.tile([P, OUT], bf16, name=f"wb{c}_{ot}_{d}",
                                    tag="wb", bufs=6)
                    nc.sync.dma_start(
                        wbd[:].rearrange("p (t n) -> p t n", t=NT),
                        w.rearrange("d (t p) o -> d p t o", p=P)[d, :, :, ot * P:(ot + 1) * P],
                    )
                    wb.append(wbd)

                ps = [psum.tile([P, CH], f32, name=f"ps0_{c}_{ot}_{d}",
                                tag=f"ps{d}", bufs=2) for d in range(D)]
                for it in range(NT):
                    for d in range(D):
                        nc.tensor.matmul(
                            ps[d][:],
                            wb[d][:, it * P:(it + 1) * P],
                            xt[:, it * CH:(it + 1) * CH],
                            start=(it == 0), stop=(it == NT - 1),
                        )

                def vout0(d, inv, _ot=ot, _c=c):
                    vt = v_tile(d, _ot, 0)
                    vcur.setdefault(d, {})[_ot] = vt
                    nc.vector.tensor_tensor(vt[:], ps[d][:], inv[:], op=Op.mult)

                normalize_unit([p[:] for p in ps], None, vout0, CH, f"f{c}_{ot}")
            return vcur

        # PE warm-up: ~5us of dummy matmuls on memset data fill the initial
        # input-DMA wait and lift the HAM clock gate to 2.4 GHz before the
        # first real matmul issues.
        wuw = pool.tile([P, P], bf16, name="wuw", tag="wuw")
        wux = pool.tile([P, 512], bf16, name="wux", tag="wux")
        nc.gpsimd.memset(wuw[:], 0.0)
        nc.gpsimd.memset(wux[:], 0.0)
        wups = psum.tile([P, 512], f32, name="wups", tag="ps0", bufs=2)
        for _ in range(24):
            nc.tensor.matmul(wups[:], wuw[:], wux[:], start=True, stop=True)

        vcur_pending = {0: first_matmul(0)}
        emit_preamble()

        for c in range(NCH):
            vcur = vcur_pending.pop(c)
            # ---------------- steps 1..STEPS-1 (normal orientation) ------
            for s in range(1, STEPS):
                vnext = {}
                for it in range(NT):
                    ps = [psum.tile([P, CH], f32, name=f"ps{s}_{c}_{it}_{d}",
                                    tag=f"ps{d}", bufs=2) for d in range(D)]
                    for j in range(NT):
                        for d in range(D):
                            nc.tensor.matmul(
                                ps[d][:],
                                ktb[j][:, it * P:(it + 1) * P],
                                vcur[d][j][:],
                                start=(j == 0), stop=(j == NT - 1),
                            )
                    bias_aps = [omc[it][:, d:d + 1] for d in range(D)]

                    def vouts(d, inv, _it=it, _s=s):
                        vt = v_tile(d, _it, _s)
                        vnext.setdefault(d, {})[_it] = vt
                        nc.vector.scalar_tensor_tensor(
                            vt[:], ps[d][:], omc[_it][:, d:d + 1], inv[:],
                            op0=Op.add, op1=Op.mult,
                        )

                    normalize_unit([p[:] for p in ps], bias_aps, vouts, CH,
                                   f"s{s}_{c}_{it}")
                vcur = vnext

            # chunk c+1's first matmul emitted here: its matmuls fill the
            # last-step drain stalls, and its v0 slots are free by now.
            if c + 1 < NCH:
                vcur_pending[c + 1] = first_matmul(c + 1)

            # ---------------- last step, transposed: u[b, i] --------------
            for bt in range(CH // P):
                # final b-tile of the final chunk: quarter-width units so the
                # end-of-kernel drain (which nothing can overlap) is shorter
                final_tile = (c == NCH - 1 and bt == CH // P - 1)
                NW = 256 if final_tile else 512
                for ic in range(OUT // NW):
                    V = pool.tile([P, NW * D], f32, name=f"V{c}_{bt}_{ic}",
                                  tag="big", bufs=3)
                    ps = [psum.tile([P, NW], f32, name=f"psL_{c}_{bt}_{ic}_{d}",
                                    tag=f"ps{d}", bufs=2) for d in range(D)]
                    for j in range(NT):
                        for d in range(D):
                            nc.tensor.matmul(
                                ps[d][:],
                                vcur[d][j][:, bt * P:(bt + 1) * P],
                                ktb[j][:, ic * NW:(ic + 1) * NW],
                                start=(j == 0),
                                stop=(omega_zero and j == NT - 1),
                            )
                    if not omega_zero:
                        for d in range(D):
                            nc.tensor.matmul(
                                ps[d][:],
                                ones1_box[0][:],
                                omrb[d][:, ic * NW:(ic + 1) * NW],
                                start=False, stop=True,
                            )

                    # stage u out of PSUM early so the banks free in ~3us
                    # instead of being held through the slow strided V writes
                    u = [pool.tile([P, NW], f32, name=f"u{d}_L{c}_{bt}_{ic}",
                                   tag=f"u{d}", bufs=2) for d in range(D)]
                    for d in range(D):
                        nc.scalar.copy(u[d][:], ps[d][:])

                    def voutL(d, inv, _V=V, _u=u, _NW=NW):
                        nc.vector.tensor_tensor(
                            _V[:, d:d + (_NW - 1) * D + 1:D], _u[d][:], inv[:],
                            op=Op.mult,
                        )

                    normalize_unit([p[:] for p in ps], None, voutL, NW,
                                   f"L{c}_{bt}_{ic}")
                    nc.sync.dma_start(
                        out[(c * CH + bt * P):(c * CH + (bt + 1) * P),
                            NW * D * ic:NW * D * (ic + 1)],
                        V[:],
                    )

    nc.finalize()
    if os.environ.get("KERNEL_FUSE") == "1":
        _fuse_ldweights_json(nc)
    elif os.environ.get("KERNEL_NODEDUP") != "1":
        _dedup_ldweights_json(nc)
    return nc


_CACHED = {}


def kernel(x, W_in, omega, coupling):
    _install_hook_shim()
    _patch_ldw_opt()
    from concourse.bass_utils import run_bass_kernel_spmd

    x = np.ascontiguousarray(np.asarray(x, dtype=np.float32))
    W_in = np.asarray(W_in, dtype=np.float32)
    omega = np.ascontiguousarray(np.asarray(omega, dtype=np.float32))
    coupling = np.asarray(coupling, dtype=np.float32)

    w_host = np.ascontiguousarray(W_in.transpose(2, 0, 1).astype(np.float16))  # [D, IN, OUT]
    ct_host = np.ascontiguousarray(coupling.T)                     # [OUT, OUT]
    omr_host = np.ascontiguousarray(omega.T)                       # [D, OUT]
    eye_host = np.eye(P, dtype=np.float32)

    omega_zero = not np.any(omega)
    key = ("nc", omega_zero)
    if key not in _CACHED:
        _CACHED[key] = _build(omega_zero)
    nc = _CACHED[key]

    in_maps = []
    for core in range(NCORES):
        xs = x[core * BS:(core + 1) * BS, :]
        in_maps.append({
            "xT": np.ascontiguousarray(xs.T.astype(np.float16)),
            "w": w_host,
            "ct": ct_host,
            "om": omega,
            "omr": omr_host,
            "eye_in": eye_host,
        })

    trace = os.environ.get("KERNEL_TRACE") == "1"
    res = run_bass_kernel_spmd(nc, in_maps, core_ids=list(range(NCORES)), trace=trace)
    if trace and res.exec_time_ns:
        print(f"HW exec time: {res.exec_time_ns} ns")
        _CACHED["exec_time_ns"] = res.exec_time_ns
        _CACHED["results"] = res

    outs = [res.results[i]["out"].reshape(BS, OUT, D) for i in range(NCORES)]
    return np.concatenate(outs, axis=0)
